# revision 1
# baseline (speedup 1.0000x reference)
"""Trainium2 Bass kernel: sparse multi-head 3x3x3 voxel conv (gnn message passing).

Self-tap (k=13) chunks use plain HWDGE DMAs (contiguous per-core shard) and
are interleaved evenly among the [128,1]-offset indirect-gather chunks so the
gpsimd queue never idles; fold is count-sorted CCE-add rounds.
"""

import sys
from contextlib import ExitStack

for p in ("/opt/trn_rl_repo", "/root/.axon_site/_ro/trn_rl_repo"):
    if p not in sys.path:
        sys.path.insert(0, p)

import numpy as np
import ml_dtypes

import concourse.tile as tile
from concourse import bass, bacc, mybir
from concourse.bass import IndirectOffsetOnAxis
from concourse.masks import make_identity

BF16 = ml_dtypes.bfloat16
C = 64
CH = 16
NH = 4
KVOL = 27
PAD_OFF = 5_000_000


def cdiv(a, b):
    return (a + b - 1) // b


def host_prep(feats, weight, kernel_map, n_cores, Q=32):
    feats = np.asarray(feats)
    weight = np.asarray(weight)
    kernel_map = np.asarray(kernel_map)
    N = feats.shape[0]
    S = N // n_cores
    ZERO_ROW = N

    table = np.zeros((N + 1, C), dtype=BF16)
    table[:N] = feats.astype(BF16)

    w_sb = np.zeros((128, KVOL * C), dtype=BF16)
    for k in range(KVOL):
        blk = np.zeros((C, C), np.float32)
        for h in range(NH):
            blk[h * CH:(h + 1) * CH, h * CH:(h + 1) * CH] = weight[k, h]
        w_sb[:C, k * C:(k + 1) * C] = blk.astype(BF16)
        w_sb[C:, k * C:(k + 1) * C] = w_sb[:C, k * C:(k + 1) * C]

    core_runs = []
    for c in range(n_cores):
        km = kernel_map[:, c * S:(c + 1) * S]
        runs = []
        for k in range(KVOL):
            m = km[k] >= 0
            runs.append((np.nonzero(m)[0].astype(np.int64),
                         km[k][m].astype(np.int64)))
        core_runs.append(runs)

    n_chunks_k = [max(cdiv(len(core_runs[c][k][0]), 128)
                      for c in range(n_cores)) for k in range(KVOL)]
    chunk_k = []
    chunk_start_k = []
    for k in range(KVOL):
        chunk_start_k.append(len(chunk_k))
        chunk_k.extend([k] * n_chunks_k[k])
    NCH_REAL = len(chunk_k)
    # pad chunk count to multiple of Q (H-batching)
    NCH = cdiv(len(chunk_k), Q) * Q
    SELF_START = chunk_start_k[13]
    N13 = n_chunks_k[13]
    # interleave self chunks (plain-DMA, no gpsimd work) evenly among
    # indirect chunks so the gpsimd gather stream never idles
    selfs = list(range(SELF_START, SELF_START + N13))
    nonself = [c for c in range(NCH_REAL) if not (SELF_START <= c < SELF_START + N13)]
    new_order = []
    acc_f = 0.0
    step = len(selfs) / max(len(nonself), 1)
    si = 0
    for c in nonself:
        new_order.append(c)
        acc_f += step
        while acc_f >= 1.0 and si < len(selfs):
            new_order.append(selfs[si]); si += 1; acc_f -= 1.0
    while si < len(selfs):
        new_order.append(selfs[si]); si += 1
    assert len(new_order) == NCH_REAL
    newpos_of = np.empty(NCH_REAL, np.int64)
    newpos_of[np.array(new_order)] = np.arange(NCH_REAL)
    chunk_k = [chunk_k[c] for c in new_order] + [0] * (NCH - NCH_REAL)
    # per-new-position: row offset into self_feats (or -1 for indirect chunks)
    self_row0 = [-1] * NCH
    for j, c in enumerate(selfs):
        self_row0[int(newpos_of[c])] = j * 128
    n_slots = NCH * 128
    NGRP = NCH // Q  # H-write groups

    # fold tiling: 128-dest tiles, count-sorted; R_t = global max per tile
    n_tiles = cdiv(S, 128)
    S_pad = n_tiles * 128
    core_counts_sorted = []
    core_orders = []
    for c in range(n_cores):
        counts = np.zeros(S, np.int64)
        for k in range(KVOL):
            counts[core_runs[c][k][0]] += 1
        order = np.argsort(-counts, kind="stable")
        core_orders.append(order)
        core_counts_sorted.append(counts[order])
    R_t = []
    for t in range(n_tiles):
        r = 0
        for c in range(n_cores):
            cs = core_counts_sorted[c]
            if t * 128 < len(cs):
                r = max(r, int(cs[t * 128]))
        R_t.append(r)
    col_base = np.concatenate([[0], np.cumsum(R_t)]).astype(np.int64)
    NR = int(col_base[-1])

    meta = dict(N=N, S=S, Q=Q, NCH=NCH, NGRP=NGRP, chunk_k=chunk_k,
                R_t=R_t, NR=NR, n_tiles=n_tiles, S_pad=S_pad,
                n_slots=n_slots, ZERO_ROW=ZERO_ROW,
                NCH_REAL=NCH_REAL, N13=N13, self_row0=self_row0)

    # h_row(s): slot s = c*128+p; group g = c//Q, q = c%Q
    # H dram row = g*128*Q + p*Q + q  (per-group partition-major, 1 desc/part)
    def h_row_of(s):
        cc = s // 128
        p = s % 128
        g = cc // Q
        q = cc % Q
        return g * 128 * Q + p * Q + q

    in_maps = []
    perms = []
    for c in range(n_cores):
        runs = core_runs[c]
        order = core_orders[c]
        rank = np.empty(S, np.int64)
        rank[order] = np.arange(S)

        gather_rows = np.full(n_slots, ZERO_ROW, np.int64)
        all_dest = []
        all_s = []
        for k in range(KVOL):
            dests, srcs = runs[k]
            L = len(dests)
            if L == 0:
                continue
            j = np.arange(L)
            s_ids = newpos_of[chunk_start_k[k] + j // 128] * 128 + (j % 128)
            gather_rows[s_ids] = srcs
            all_dest.append(dests)
            all_s.append(s_ids)
        all_dest = np.concatenate(all_dest)
        all_hrow = h_row_of(np.concatenate(all_s))

        goffs = np.ascontiguousarray(
            gather_rows.reshape(NCH, 128).T, dtype=np.int32)  # [128, NCH]

        pr = rank[all_dest]
        o2 = np.argsort(pr, kind="stable")
        sr = pr[o2]
        sh = all_hrow[o2]
        grp_start = np.searchsorted(sr, np.arange(S))
        r_idx = np.arange(len(sr)) - grp_start[sr]
        t_of = sr // 128
        p_of = sr % 128
        col = col_base[t_of] + r_idx
        assert (r_idx < np.array(R_t)[t_of]).all()
        foffs = np.full((128, NR), PAD_OFF, np.int32)
        foffs[p_of, col] = sh
        # round 0 fully initializes real dests (self tap); pad dest slots in
        # the final partial tile stay PAD -> skipped -> stale rows dropped on host.

        self_feats = np.zeros((N13 * 128, C), dtype=BF16)
        self_feats[:S] = table[c * S:(c + 1) * S]
        in_maps.append({
            "table": table,
            "w_sb": w_sb,
            "goffs": goffs,
            "foffs": foffs,
            "self_feats": self_feats,
        })
        perms.append(order)

    return in_maps, perms, meta


def build_program(n_cores, meta):
    Q, NCH, NGRP = meta["Q"], meta["NCH"], meta["NGRP"]
    chunk_k, R_t = meta["chunk_k"], meta["R_t"]
    n_tiles, S_pad, N = meta["n_tiles"], meta["S_pad"], meta["N"]
    n_H_rows = meta["n_slots"]

    nc = bacc.Bacc("TRN2", target_bir_lowering=False, debug=False,
                   num_devices=n_cores)

    table = nc.dram_tensor("table", [N + 1, C], mybir.dt.bfloat16,
                           kind="ExternalInput").ap()
    w_in = nc.dram_tensor("w_sb", [128, KVOL * C], mybir.dt.bfloat16,
                          kind="ExternalInput").ap()
    goffs = nc.dram_tensor("goffs", [128, NCH], mybir.dt.int32,
                           kind="ExternalInput").ap()
    foffs = nc.dram_tensor("foffs", [128, meta["NR"]], mybir.dt.int32,
                           kind="ExternalInput").ap()
    self_in = nc.dram_tensor("self_feats", [meta["N13"] * 128, C],
                             mybir.dt.bfloat16, kind="ExternalInput").ap()
    out = nc.dram_tensor("out", [S_pad, C], mybir.dt.float32,
                         kind="ExternalOutput").ap()

    with tile.TileContext(nc) as tc, ExitStack() as ctx:
        dram = ctx.enter_context(tc.tile_pool(name="dram", bufs=1, space="DRAM"))
        h_dram = dram.tile([n_H_rows, C], mybir.dt.bfloat16)

        wpool = ctx.enter_context(tc.tile_pool(name="w", bufs=1))
        w_t = wpool.tile([128, KVOL * C], mybir.dt.bfloat16)
        nc.sync.dma_start(out=w_t[:], in_=w_in[:])
        ident = wpool.tile([128, 128], mybir.dt.bfloat16)
        make_identity(nc, ident[:])
        gof = wpool.tile([128, NCH], mybir.dt.int32)
        nc.sync.dma_start(out=gof[:], in_=goffs[:])
        fof = wpool.tile([128, meta["NR"]], mybir.dt.int32)
        nc.sync.dma_start(out=fof[:], in_=foffs[:])

        gp = ctx.enter_context(tc.tile_pool(name="G", bufs=8))
        xp = ctx.enter_context(tc.tile_pool(name="X", bufs=8))
        hp = ctx.enter_context(tc.tile_pool(name="H", bufs=3))
        psx = ctx.enter_context(tc.tile_pool(name="psx", bufs=4, space="PSUM"))
        psh = ctx.enter_context(tc.tile_pool(name="psh", bufs=4, space="PSUM"))

        self_row0 = meta["self_row0"]
        NCH_REAL = meta["NCH_REAL"]
        for g in range(NGRP):
            h_t = hp.tile([128, Q * C], mybir.dt.bfloat16)
            for qi in range(Q):
                cid = g * Q + qi
                if cid >= NCH_REAL:
                    continue  # all-pad tail chunk: H garbage, never referenced
                k = chunk_k[cid]
                g1 = gp.tile([128, C], mybir.dt.bfloat16)
                if self_row0[cid] >= 0:
                    # self tap: sources are this core's own contiguous shard
                    j0 = self_row0[cid]
                    nc.sync.dma_start(out=g1[:], in_=self_in[j0:j0 + 128, :])
                else:
                    nc.gpsimd.indirect_dma_start(
                        out=g1[:], out_offset=None,
                        in_=table[:],
                        in_offset=IndirectOffsetOnAxis(
                            ap=gof[:, cid:cid + 1], axis=0),
                    )
                x_ps = psx.tile([64, 128], mybir.dt.bfloat16)
                nc.tensor.transpose(out=x_ps[:], in_=g1[:], identity=ident[:])
                x_t = xp.tile([64, 128], mybir.dt.bfloat16)
                nc.vector.tensor_copy(out=x_t[:], in_=x_ps[:])
                h_ps = psh.tile([128, C], mybir.dt.float32)
                nc.tensor.matmul(
                    out=h_ps[:],
                    lhsT=x_t[:],
                    rhs=w_t[0:64, k * C:(k + 1) * C],
                    start=True, stop=True,
                )
                nc.scalar.activation(
                    h_t[:, qi * C:(qi + 1) * C], h_ps[:],
                    mybir.ActivationFunctionType.Copy,
                )
            nc.sync.dma_start(
                out=h_dram[g * 128 * Q:(g + 1) * 128 * Q, :].rearrange(
                    "(p q) c -> p (q c)", p=128),
                in_=h_t[:],
            )

        fop = ctx.enter_context(tc.tile_pool(name="acc", bufs=4))
        outp = ctx.enter_context(tc.tile_pool(name="outp", bufs=4))
        col = 0
        for t in range(n_tiles):
            acc = fop.tile([128, C], mybir.dt.bfloat16)
            for r in range(R_t[t]):
                nc.gpsimd.indirect_dma_start(
                    out=acc[:], out_offset=None,
                    in_=h_dram[:],
                    in_offset=IndirectOffsetOnAxis(
                        ap=fof[:, col:col + 1], axis=0),
                    compute_op=(mybir.AluOpType.bypass if r == 0
                                else mybir.AluOpType.add),
                    bounds_check=n_H_rows - 1,
                    oob_is_err=False,
                )
                col += 1
            out_t = outp.tile([128, C], mybir.dt.float32)
            nc.vector.tensor_copy(out=out_t[:], in_=acc[:])
            nc.sync.dma_start(out=out[t * 128:(t + 1) * 128, :], in_=out_t[:])

    nc.compile()
    return nc


def assemble_output(results, perms, meta, n_cores):
    S = meta["S"]
    N = meta["N"]
    out = np.empty((N, C), np.float32)
    for c in range(n_cores):
        rows = results[c]["out"]
        out[c * S + perms[c]] = rows[:S]
    return out


N_CORES = 8
LAST_EXEC_TIME_NS = None

_CACHE = {}


def kernel(feats, weight, kernel_map):
    """Full-input entry point: shard, run on 8 NeuronCores, unshard."""
    global LAST_EXEC_TIME_NS
    import os
    from concourse import bass_utils

    feats = np.asarray(feats)
    weight = np.asarray(weight)
    kernel_map = np.asarray(kernel_map)

    in_maps, perms, meta = host_prep(feats, weight, kernel_map, N_CORES, Q=32)
    key = (meta["NCH"], meta["NR"], tuple(meta["R_t"][:4]))
    if key in _CACHE:
        nc = _CACHE[key]
    else:
        nc = build_program(N_CORES, meta)
        _CACHE[key] = nc

    trace = os.environ.get("BASS_KERNEL_TRACE", "0") == "1"
    res = bass_utils.run_bass_kernel_spmd(
        nc, in_maps, core_ids=list(range(N_CORES)), trace=trace)
    LAST_EXEC_TIME_NS = res.exec_time_ns
    return assemble_output(res.results, perms, meta, N_CORES)



# revision 5
# speedup vs baseline: 1.4958x; 1.4958x over previous
"""Trainium2 Bass kernel: sparse multi-head 3x3x3 voxel conv (gnn message passing).

v3: the kernel is Pool(gpsimd)-bound — every data-dependent row move costs
~6.5-11ns of Q7 descriptor-generation time. This version minimizes Q7 work:
  - gather: InstDMAGatherAnt (dma_gather, ~6.5ns/row) from an f32 table,
    pairs sorted (table-segment, k, src) so each op is segment-pure
    (int16 idx limit) and covers up to 4096 rows.
  - transform: paired PE transposes (f32) + block-diag 128-wide matmuls
    (bf16); k-boundary pairs use two half-matmuls.
  - self tap (k=13): zero Q7 — plain DMA from a rank-ordered image; its
    fold contribution is read back with plain strided DMA (r=0 block).
  - fold: per-column [128,1]-offset indirect DMAs (~1.4us each) +
    DVE tensor_reduce; only non-self columns (~2.1k ops).
"""

import sys
from contextlib import ExitStack

for p in ("/opt/trn_rl_repo", "/root/.axon_site/_ro/trn_rl_repo"):
    if p not in sys.path:
        sys.path.insert(0, p)

import numpy as np
import ml_dtypes

import concourse.tile as tile
from concourse import bass, bacc, mybir
from concourse.bass import IndirectOffsetOnAxis
from concourse.masks import make_identity

BF16 = ml_dtypes.bfloat16
C = 64
CH = 16
NH = 4
KVOL = 27
N_CORES = 8
SEGR = 32768        # table rows per int16 segment
Q_IND = 32          # indirect chunks per batch (g_t f32: 8KB/partition)
Q_SELF = 64         # self chunks per batch (bf16)
QF = 64             # fold columns buffered per f_t tile
TMAX = 16           # max dest tiles per fold group
MAXI = 4096         # max idxs per dma_gather op


def cdiv(a, b):
    return (a + b - 1) // b


def host_prep(feats, weight, kernel_map, n_cores):
    feats = np.asarray(feats)
    weight = np.asarray(weight)
    kernel_map = np.asarray(kernel_map)
    N = feats.shape[0]
    S = N // n_cores
    n_segs = cdiv(N + 1, SEGR)

    table32 = np.zeros((N + 1, C), dtype=np.float32)
    table32[:N] = np.asarray(feats.astype(BF16), dtype=np.float32)
    table_bf = table32.astype(BF16)

    # w2[:, k*128:(k+1)*128] = diag(Wk, Wk); halves also usable separately
    w2 = np.zeros((128, KVOL * 128), dtype=BF16)
    for k in range(KVOL):
        blk = np.zeros((C, C), np.float32)
        for h in range(NH):
            blk[h * CH:(h + 1) * CH, h * CH:(h + 1) * CH] = weight[k, h]
        w2[:C, k * 128:k * 128 + C] = blk.astype(BF16)
        w2[C:, k * 128 + C:(k + 1) * 128] = blk.astype(BF16)

    self_k = 13 if bool((kernel_map[13] == np.arange(N)).all()) else None
    assert self_k is not None, "expected identity self tap"
    ind_ks = [k for k in range(KVOL) if k != self_k]

    # per-core pair runs and counts
    core_runs = []
    core_orders = []
    core_counts_sorted = []
    for c in range(n_cores):
        km = kernel_map[:, c * S:(c + 1) * S]
        runs = {}
        counts = np.zeros(S, np.int64)
        for k in range(KVOL):
            m = km[k] >= 0
            counts += m
            if k == self_k:
                continue
            runs[k] = (np.nonzero(m)[0].astype(np.int64),
                       km[k][m].astype(np.int64))
        core_runs.append(runs)
        order = np.argsort(-counts, kind="stable")
        core_orders.append(order)
        core_counts_sorted.append(counts[order])

    # group sizes: (seg, k) chunk counts = max over cores, so the chunk
    # structure is identical on every core (SPMD single program)
    grp_chunks = {}
    for seg in range(n_segs):
        for k in ind_ks:
            mx = 0
            for c in range(n_cores):
                dests, srcs = core_runs[c][k]
                cnt = int(((srcs >= seg * SEGR) & (srcs < (seg + 1) * SEGR)).sum())
                mx = max(mx, cnt)
            if mx:
                grp_chunks[(seg, k)] = cdiv(mx, 128)

    # global chunk stream: (seg, k) groups in seg-major order
    chunk_meta = []          # per chunk: (seg, k)
    grp_start = {}
    for seg in range(n_segs):
        for k in ind_ks:
            nch = grp_chunks.get((seg, k), 0)
            if not nch:
                continue
            grp_start[(seg, k)] = len(chunk_meta)
            chunk_meta.extend([(seg, k)] * nch)
    NCHI = len(chunk_meta)

    N13 = cdiv(S, 128)
    n_tiles = cdiv(S, 128)
    S_pad = n_tiles * 128

    # batches: runs of chunks (ind: Q_IND, self: Q_SELF), interleaved
    def split(total, q, kind):
        out = []
        pos = 0
        while pos < total:
            qc = min(q, total - pos)
            out.append(dict(kind=kind, qcount=qc, col=pos))
            pos += qc
        return out

    bi = split(NCHI, Q_IND, "ind")
    bs = split(N13, Q_SELF, "self")
    batches = []
    step = len(bi) / max(len(bs), 1)
    acc = 0.0
    si = 0
    for b in bi:
        batches.append(b)
        acc += 1.0
        while acc >= step and si < len(bs):
            batches.append(bs[si]); si += 1; acc -= step
    while si < len(bs):
        batches.append(bs[si]); si += 1

    hbase = 0
    for b in batches:
        b["hbase"] = hbase
        hbase += 128 * b["qcount"]
    n_H_rows = hbase
    ZH = n_H_rows

    # gather ops: per ind batch, split its chunk run at segment boundaries
    # and at MAXI idxs. op = dict(b, c0(chunk offset in batch), nch, seg)
    for b in batches:
        if b["kind"] != "ind":
            continue
        ops = []
        c0 = 0
        while c0 < b["qcount"]:
            seg = chunk_meta[b["col"] + c0][0]
            n = 0
            while (c0 + n < b["qcount"]
                   and chunk_meta[b["col"] + c0 + n][0] == seg
                   and (n + 1) * 128 <= MAXI):
                n += 1
            ops.append(dict(c0=c0, nch=n, seg=seg))
            c0 += n
        b["ops"] = ops

    # H row of (ind chunk, p) / (self chunk, p)
    ind_chunk_batch = np.empty(max(NCHI, 1), np.int64)
    ind_chunk_q = np.empty(max(NCHI, 1), np.int64)
    self_chunk_batch = np.empty(max(N13, 1), np.int64)
    self_chunk_q = np.empty(max(N13, 1), np.int64)
    for idx, b in enumerate(batches):
        sl = slice(b["col"], b["col"] + b["qcount"])
        if b["kind"] == "ind":
            ind_chunk_batch[sl] = idx
            ind_chunk_q[sl] = np.arange(b["qcount"])
        else:
            self_chunk_batch[sl] = idx
            self_chunk_q[sl] = np.arange(b["qcount"])
    bat_hbase = np.array([b["hbase"] for b in batches], np.int64)
    bat_qc = np.array([b["qcount"] for b in batches], np.int64)

    # fold tiling: count-sorted dests; R_t = global max count per tile
    # (block 0 = self, read via plain DMA; fof covers r>=1 only)
    R_t = []
    for t in range(n_tiles):
        r = 1
        for c in range(n_cores):
            cs = core_counts_sorted[c]
            if t * 128 < len(cs):
                r = max(r, int(cs[t * 128]))
        R_t.append(r)
    colb = np.concatenate([[0], np.cumsum(np.maximum(np.array(R_t) - 1, 0))])
    NRF = int(colb[-1])      # fof columns (non-self)

    fold_ops = []
    t = 0
    while t < n_tiles:
        t0 = t
        ncols = 0
        while t < n_tiles and t - t0 < TMAX and ncols + R_t[t] <= QF:
            ncols += R_t[t]
            t += 1
        assert t > t0
        fold_ops.append(dict(t0=t0, ntiles=t - t0))

    meta = dict(N=N, S=S, NCHI=NCHI, N13=N13, batches=batches,
                chunk_meta=chunk_meta, n_H_rows=n_H_rows, ZH=ZH,
                R_t=R_t, NRF=NRF, n_tiles=n_tiles, S_pad=S_pad,
                fold_ops=fold_ops, n_segs=n_segs,
                self_chunk_batch=[int(x) for x in
                                  self_chunk_batch[:max(N13, 1)]],
                self_chunk_q=[int(x) for x in self_chunk_q[:max(N13, 1)]])

    # per-core data
    in_maps = []
    for c in range(n_cores):
        order = core_orders[c]
        rank = np.empty(S, np.int64)
        rank[order] = np.arange(S)

        # slot assignment per (seg, k) group; idx stream per chunk
        gidx_rows = np.full((NCHI, 128), -1, np.int64)  # abs table row
        all_rank = []
        all_hrow = []
        for k in ind_ks:
            dests, srcs = core_runs[c][k]
            if len(dests) == 0:
                continue
            seg_of = srcs // SEGR
            for seg in range(n_segs):
                m = seg_of == seg
                L = int(m.sum())
                if L == 0:
                    continue
                g0 = grp_start[(seg, k)]
                d_g = dests[m]
                s_g = srcs[m]
                j = np.arange(L)
                ci = g0 + j // 128
                p = j % 128
                gidx_rows[ci, p] = s_g
                all_rank.append(rank[d_g])
                bidx = ind_chunk_batch[ci]
                all_hrow.append(bat_hbase[bidx] + p * bat_qc[bidx]
                                + ind_chunk_q[ci])
        all_rank = np.concatenate(all_rank)
        all_hrow = np.concatenate(all_hrow)

        # pad slots -> any valid row of the chunk's segment (unreferenced)
        for ci in range(NCHI):
            seg = chunk_meta[ci][0]
            mpad = gidx_rows[ci] < 0
            gidx_rows[ci, mpad] = seg * SEGR

        # idx stream, int16 relative to segment, 16-wrapped + replicated:
        # op covers chunks [b.col+c0, +nch): idx j (chunk-local run) at
        # wrapped [16r + j%16, j//16]
        gidx = np.zeros((128, NCHI * 8), np.int16)
        for ci in range(NCHI):
            seg = chunk_meta[ci][0]
            rel = (gidx_rows[ci] - seg * SEGR).astype(np.int16)
            w = rel.reshape(8, 16).T          # [16, 8]
            gidx[:, ci * 8:(ci + 1) * 8] = np.tile(w, (8, 1))

        # fold offsets (non-self contributions, r>=1)
        o2 = np.argsort(all_rank, kind="stable")
        sr = all_rank[o2]
        sh = all_hrow[o2]
        grp_s = np.searchsorted(sr, np.arange(S))
        r_idx = np.arange(len(sr)) - grp_s[sr]
        t_of = sr // 128
        p_of = sr % 128
        col = colb[t_of] + r_idx
        assert (r_idx < np.array(R_t)[t_of] - 1).all(), "count excl self"
        fof = np.full((128, max(NRF, 1)), ZH, np.int32)
        fof[p_of, col] = sh

        # self image in dest-RANK order: chunk j slot p = dest order[j*128+p]
        d = np.arange(N13 * 128)
        src_local = np.where(d < S, order[np.minimum(d, S - 1)], 0)
        vals = table_bf[np.where(d < S, c * S + src_local, N)]  # [N13*128, C]
        self_img = np.ascontiguousarray(
            vals.reshape(N13, 128, C).transpose(1, 0, 2).reshape(128, N13 * C))

        in_maps.append({
            "table32": table32,
            "w2": w2,
            "gidx": gidx,
            "fof": fof,
            "self_img": self_img,
        })

    return in_maps, core_orders, meta


def build_program(n_cores, meta):
    NCHI, N13 = meta["NCHI"], meta["N13"]
    batches = meta["batches"]
    chunk_meta = meta["chunk_meta"]
    n_tiles = meta["n_tiles"]
    N = meta["N"]
    n_H_rows = meta["n_H_rows"]
    fold_ops = meta["fold_ops"]
    R_t = meta["R_t"]
    NRF = meta["NRF"]
    n_segs = meta["n_segs"]
    s_cb = meta["self_chunk_batch"]
    s_cq = meta["self_chunk_q"]
    colb = np.concatenate([[0], np.cumsum(np.maximum(np.array(R_t) - 1, 0))])

    nc = bacc.Bacc("TRN2", target_bir_lowering=False, debug=False,
                   num_devices=n_cores)

    table = nc.dram_tensor("table32", [N + 1, C], mybir.dt.float32,
                           kind="ExternalInput").ap()
    w_in = nc.dram_tensor("w2", [128, KVOL * 128], mybir.dt.bfloat16,
                          kind="ExternalInput").ap()
    gidx_in = nc.dram_tensor("gidx", [128, NCHI * 8], mybir.dt.int16,
                             kind="ExternalInput").ap()
    fof_in = nc.dram_tensor("fof", [128, max(NRF, 1)], mybir.dt.int32,
                            kind="ExternalInput").ap()
    self_in = nc.dram_tensor("self_img", [128, max(N13, 1) * C],
                             mybir.dt.bfloat16, kind="ExternalInput").ap()
    out = nc.dram_tensor("out_img", [128, n_tiles * C], mybir.dt.float32,
                         kind="ExternalOutput").ap()

    with tile.TileContext(nc) as tc, ExitStack() as ctx:
        dram = ctx.enter_context(tc.tile_pool(name="dram", bufs=1, space="DRAM"))
        h_dram = dram.tile([n_H_rows + 128, C], mybir.dt.bfloat16)

        wp = ctx.enter_context(tc.tile_pool(name="w", bufs=1))
        w_t = wp.tile([128, KVOL * 128], mybir.dt.bfloat16)
        nc.sync.dma_start(out=w_t[:], in_=w_in[:])
        ident = wp.tile([128, 128], mybir.dt.bfloat16)
        make_identity(nc, ident[:])
        identf = wp.tile([128, 128], mybir.dt.float32)
        nc.vector.tensor_copy(out=identf[:], in_=ident[:])
        gix = wp.tile([128, NCHI * 8], mybir.dt.int16)
        nc.sync.dma_start(out=gix[:], in_=gidx_in[:])
        fof = wp.tile([128, max(NRF, 1)], mybir.dt.int32)
        nc.sync.dma_start(out=fof[:], in_=fof_in[:])
        zt = wp.tile([128, C], mybir.dt.bfloat16)
        nc.vector.memset(zt[:], 0.0)
        nc.sync.dma_start(out=h_dram[n_H_rows:n_H_rows + 128, :], in_=zt[:])

        gp = ctx.enter_context(tc.tile_pool(name="G", bufs=3))
        xp = ctx.enter_context(tc.tile_pool(name="X", bufs=4))
        hp = ctx.enter_context(tc.tile_pool(name="H", bufs=3))
        psx = ctx.enter_context(tc.tile_pool(name="psx", bufs=3, space="PSUM"))
        psh = ctx.enter_context(tc.tile_pool(name="psh", bufs=3, space="PSUM"))

        def pair_pipeline(g_t, h_t, qc, ks, fdtype):
            npairs = qc // 2
            odd = qc % 2
            idf = identf if fdtype == mybir.dt.float32 else ident
            for g0 in range(0, npairs + odd, 4):
                gn = min(4, npairs + odd - g0)
                x_ps = psx.tile([128, 512], fdtype)
                if odd and g0 + gn == npairs + odd:
                    nc.vector.memset(x_ps[:], 0.0)
                for j in range(gn):
                    pr = g0 + j
                    if pr < npairs:
                        nc.tensor.transpose(
                            out=x_ps[:, j * 128:(j + 1) * 128],
                            in_=g_t[:, pr * 128:(pr + 1) * 128],
                            identity=idf[:])
                    else:
                        nc.tensor.transpose(
                            out=x_ps[0:64, j * 128:(j + 1) * 128],
                            in_=g_t[:, pr * 128:pr * 128 + 64],
                            identity=idf[:])
                x_t = xp.tile([128, 512], mybir.dt.bfloat16)
                nc.vector.tensor_copy(out=x_t[:, :gn * 128],
                                      in_=x_ps[:, :gn * 128])
                h_ps = psh.tile([128, 512], mybir.dt.float32)
                for j in range(gn):
                    pr = g0 + j
                    if pr < npairs:
                        k0, k1 = ks[2 * pr], ks[2 * pr + 1]
                        if k0 == k1:
                            nc.tensor.matmul(
                                out=h_ps[:, j * 128:(j + 1) * 128],
                                lhsT=x_t[:, j * 128:(j + 1) * 128],
                                rhs=w_t[:, k0 * 128:(k0 + 1) * 128],
                                start=True, stop=True)
                        else:
                            nc.tensor.matmul(
                                out=h_ps[:, j * 128:j * 128 + 64],
                                lhsT=x_t[0:64, j * 128:(j + 1) * 128],
                                rhs=w_t[0:64, k0 * 128:k0 * 128 + 64],
                                start=True, stop=True)
                            nc.tensor.matmul(
                                out=h_ps[:, j * 128 + 64:(j + 1) * 128],
                                lhsT=x_t[64:128, j * 128:(j + 1) * 128],
                                rhs=w_t[64:128, k1 * 128 + 64:(k1 + 1) * 128],
                                start=True, stop=True)
                    else:  # odd tail: single chunk in low half
                        k0 = ks[2 * pr]
                        nc.tensor.matmul(
                            out=h_ps[:, j * 128:j * 128 + 64],
                            lhsT=x_t[0:64, j * 128:(j + 1) * 128],
                            rhs=w_t[0:64, k0 * 128:k0 * 128 + 64],
                            start=True, stop=True)
                wcols = min(gn * 128, qc * 64 - g0 * 128)
                nc.scalar.activation(
                    h_t[:, g0 * 128:g0 * 128 + wcols],
                    h_ps[:, :wcols],
                    mybir.ActivationFunctionType.Copy)

        for b in batches:
            qc = b["qcount"]
            if b["kind"] == "ind":
                g_t = gp.tile([128, qc * C], mybir.dt.float32)
                for op in b["ops"]:
                    ni = op["nch"] * 128
                    seg = op["seg"]
                    seg_rows = min(SEGR, (N + 1) - seg * SEGR)
                    cbase = b["col"] + op["c0"]
                    nc.gpsimd.dma_gather(
                        out_ap=g_t[:, op["c0"] * C:(op["c0"] + op["nch"]) * C]
                            .rearrange("p (c e) -> p c e", c=op["nch"]),
                        in_ap=table[seg * SEGR:seg * SEGR + seg_rows, :],
                        idxs_ap=gix[:, cbase * 8:(cbase + op["nch"]) * 8],
                        num_idxs=ni, num_idxs_reg=ni, elem_size=C,
                        single_packet=ni <= 1024)
                ks = [chunk_meta[b["col"] + q][1] for q in range(qc)]
                fdtype = mybir.dt.float32
            else:
                g_t = gp.tile([128, qc * C], mybir.dt.bfloat16)
                nc.sync.dma_start(
                    out=g_t[:],
                    in_=self_in[:, b["col"] * C:(b["col"] + qc) * C])
                ks = [13] * qc
                fdtype = mybir.dt.bfloat16
            h_t = hp.tile([128, qc * C], mybir.dt.bfloat16)
            pair_pipeline(g_t, h_t, qc, ks, fdtype)
            nc.sync.dma_start(
                out=h_dram[b["hbase"]:b["hbase"] + 128 * qc, :].rearrange(
                    "(p q) c -> p (q c)", p=128),
                in_=h_t[:])

        fp = ctx.enter_context(tc.tile_pool(name="F", bufs=3))
        op_ = ctx.enter_context(tc.tile_pool(name="O", bufs=3))
        for fo in fold_ops:
            nt = fo["ntiles"]
            ncols = sum(R_t[fo["t0"] + i] for i in range(nt))
            f_t = fp.tile([128, ncols * C], mybir.dt.bfloat16)
            lc = 0
            for lt in range(nt):
                t = fo["t0"] + lt
                R = R_t[t]
                # block 0: self contribution, plain strided read
                bidx = s_cb[t]
                qcb = batches[bidx]["qcount"]
                hb = batches[bidx]["hbase"]
                qq = s_cq[t]
                nc.sync.dma_start(
                    out=f_t[:, lc * C:(lc + 1) * C],
                    in_=h_dram[hb:hb + 128 * qcb, :].rearrange(
                        "(p q) c -> p q c", p=128)[:, qq, :])
                # blocks 1..R-1: indirect per column
                for r in range(R - 1):
                    colx = int(colb[t]) + r
                    nc.gpsimd.indirect_dma_start(
                        out=f_t[:, (lc + 1 + r) * C:(lc + 2 + r) * C],
                        out_offset=None,
                        in_=h_dram[:],
                        in_offset=IndirectOffsetOnAxis(
                            ap=fof[:, colx:colx + 1], axis=0),
                    )
                lc += R
            ob = op_.tile([128, nt * C], mybir.dt.float32)
            lc = 0
            for lt in range(nt):
                R = R_t[fo["t0"] + lt]
                if R == 1:
                    nc.vector.tensor_copy(
                        out=ob[:, lt * C:(lt + 1) * C],
                        in_=f_t[:, lc * C:(lc + 1) * C])
                else:
                    nc.vector.tensor_reduce(
                        out=ob[:, lt * C:(lt + 1) * C],
                        in_=f_t[:, lc * C:(lc + R) * C].rearrange(
                            "p (r c) -> p c r", r=R),
                        axis=mybir.AxisListType.X,
                        op=mybir.AluOpType.add)
                lc += R
            nc.sync.dma_start(
                out=out[:, fo["t0"] * C:(fo["t0"] + nt) * C],
                in_=ob[:])

    nc.compile()
    return nc


def assemble_output(results, orders, meta, n_cores):
    S = meta["S"]
    N = meta["N"]
    n_tiles = meta["n_tiles"]
    out = np.empty((N, C), np.float32)
    for c in range(n_cores):
        img = results[c]["out_img"].reshape(128, n_tiles, C)
        rows = np.moveaxis(img, 0, 1).reshape(n_tiles * 128, C)
        out[c * S + orders[c]] = rows[:S]
    return out


LAST_EXEC_TIME_NS = None
_CACHE = {}


def kernel(feats, weight, kernel_map):
    """Full-input entry point: shard, run on 8 NeuronCores, unshard."""
    global LAST_EXEC_TIME_NS
    import os
    from concourse import bass_utils

    feats = np.asarray(feats)
    weight = np.asarray(weight)
    kernel_map = np.asarray(kernel_map)

    in_maps, orders, meta = host_prep(feats, weight, kernel_map, N_CORES)
    key = (meta["NCHI"], meta["N13"], meta["NRF"], tuple(meta["R_t"][:8]),
           len(meta["batches"]), len(meta["fold_ops"]))
    if key in _CACHE:
        nc = _CACHE[key]
    else:
        nc = build_program(N_CORES, meta)
        _CACHE[key] = nc

    trace = os.environ.get("BASS_KERNEL_TRACE", "0") == "1"
    res = bass_utils.run_bass_kernel_spmd(
        nc, in_maps, core_ids=list(range(N_CORES)), trace=trace)
    LAST_EXEC_TIME_NS = res.exec_time_ns
    return assemble_output(res.results, orders, meta, N_CORES)


# revision 6
# speedup vs baseline: 1.5469x; 1.0342x over previous
"""Trainium2 Bass kernel: sparse multi-head 3x3x3 voxel conv (gnn message passing).

v3: the kernel is Pool(gpsimd)-bound — every data-dependent row move costs
~6.5-11ns of Q7 descriptor-generation time. This version minimizes Q7 work:
  - gather: InstDMAGatherAnt (dma_gather, ~6.5ns/row) from an f32 table,
    pairs sorted (table-segment, k, src) so each op is segment-pure
    (int16 idx limit) and covers up to 4096 rows.
  - transform: paired PE transposes (f32) + block-diag 128-wide matmuls
    (bf16); k-boundary pairs use two half-matmuls.
  - self tap (k=13): zero Q7 — plain DMA from a rank-ordered image; its
    fold contribution is read back with plain strided DMA (r=0 block).
  - fold: per-column [128,1]-offset indirect DMAs (~1.4us each) +
    DVE tensor_reduce; only non-self columns (~2.1k ops).
"""

import sys
from contextlib import ExitStack

for p in ("/opt/trn_rl_repo", "/root/.axon_site/_ro/trn_rl_repo"):
    if p not in sys.path:
        sys.path.insert(0, p)

import numpy as np
import ml_dtypes

import concourse.tile as tile
from concourse import bass, bacc, mybir
from concourse.bass import IndirectOffsetOnAxis
from concourse.masks import make_identity

BF16 = ml_dtypes.bfloat16
C = 64
CH = 16
NH = 4
KVOL = 27
N_CORES = 8
SEGR = 32768        # table rows per int16 segment
Q_IND = 32          # indirect chunks per batch (g_t f32: 8KB/partition)
Q_SELF = 64         # self chunks per batch (bf16)
QF = 64             # fold columns buffered per f_t tile
TMAX = 16           # max dest tiles per fold group
MAXI = 4096         # max idxs per dma_gather op


def cdiv(a, b):
    return (a + b - 1) // b


def host_prep(feats, weight, kernel_map, n_cores):
    feats = np.asarray(feats)
    weight = np.asarray(weight)
    kernel_map = np.asarray(kernel_map)
    N = feats.shape[0]
    S = N // n_cores
    n_segs = cdiv(N + 1, SEGR)

    table32 = np.zeros((N + 1, C), dtype=np.float32)
    table32[:N] = np.asarray(feats.astype(BF16), dtype=np.float32)
    table_bf = table32.astype(BF16)

    # w2[:, k*128:(k+1)*128] = diag(Wk, Wk); halves also usable separately
    w2 = np.zeros((128, KVOL * 128), dtype=BF16)
    for k in range(KVOL):
        blk = np.zeros((C, C), np.float32)
        for h in range(NH):
            blk[h * CH:(h + 1) * CH, h * CH:(h + 1) * CH] = weight[k, h]
        w2[:C, k * 128:k * 128 + C] = blk.astype(BF16)
        w2[C:, k * 128 + C:(k + 1) * 128] = blk.astype(BF16)

    self_k = 13 if bool((kernel_map[13] == np.arange(N)).all()) else None
    assert self_k is not None, "expected identity self tap"
    ind_ks = [k for k in range(KVOL) if k != self_k]

    # per-core pair runs and counts
    core_runs = []
    core_orders = []
    core_counts_sorted = []
    for c in range(n_cores):
        km = kernel_map[:, c * S:(c + 1) * S]
        runs = {}
        counts = np.zeros(S, np.int64)
        for k in range(KVOL):
            m = km[k] >= 0
            counts += m
            if k == self_k:
                continue
            runs[k] = (np.nonzero(m)[0].astype(np.int64),
                       km[k][m].astype(np.int64))
        core_runs.append(runs)
        order = np.argsort(-counts, kind="stable")
        core_orders.append(order)
        core_counts_sorted.append(counts[order])

    # group sizes: (seg, k) chunk counts = max over cores, so the chunk
    # structure is identical on every core (SPMD single program)
    grp_chunks = {}
    for seg in range(n_segs):
        for k in ind_ks:
            mx = 0
            for c in range(n_cores):
                dests, srcs = core_runs[c][k]
                cnt = int(((srcs >= seg * SEGR) & (srcs < (seg + 1) * SEGR)).sum())
                mx = max(mx, cnt)
            if mx:
                grp_chunks[(seg, k)] = cdiv(mx, 128)

    # global chunk stream: (seg, k) groups in seg-major order
    chunk_meta = []          # per chunk: (seg, k)
    grp_start = {}
    for seg in range(n_segs):
        for k in ind_ks:
            nch = grp_chunks.get((seg, k), 0)
            if not nch:
                continue
            grp_start[(seg, k)] = len(chunk_meta)
            chunk_meta.extend([(seg, k)] * nch)
    NCHI = len(chunk_meta)

    N13 = cdiv(S, 128)
    n_tiles = cdiv(S, 128)
    S_pad = n_tiles * 128

    # batches: runs of chunks (ind: Q_IND, self: Q_SELF), interleaved
    def split(total, q, kind):
        out = []
        pos = 0
        while pos < total:
            qc = min(q, total - pos)
            out.append(dict(kind=kind, qcount=qc, col=pos))
            pos += qc
        return out

    bi = split(NCHI, Q_IND, "ind")
    bs = split(N13, Q_SELF, "self")
    batches = []
    step = len(bi) / max(len(bs), 1)
    acc = 0.0
    si = 0
    for b in bi:
        batches.append(b)
        acc += 1.0
        while acc >= step and si < len(bs):
            batches.append(bs[si]); si += 1; acc -= step
    while si < len(bs):
        batches.append(bs[si]); si += 1

    hbase = 0
    for b in batches:
        b["hbase"] = hbase
        hbase += 128 * b["qcount"]
    n_H_rows = hbase
    ZH = n_H_rows

    # gather ops: per ind batch, split its chunk run at segment boundaries
    # and at MAXI idxs. op = dict(b, c0(chunk offset in batch), nch, seg)
    for b in batches:
        if b["kind"] != "ind":
            continue
        ops = []
        c0 = 0
        while c0 < b["qcount"]:
            seg = chunk_meta[b["col"] + c0][0]
            n = 0
            while (c0 + n < b["qcount"]
                   and chunk_meta[b["col"] + c0 + n][0] == seg
                   and (n + 1) * 128 <= MAXI):
                n += 1
            ops.append(dict(c0=c0, nch=n, seg=seg))
            c0 += n
        b["ops"] = ops

    # H row of (ind chunk, p) / (self chunk, p)
    ind_chunk_batch = np.empty(max(NCHI, 1), np.int64)
    ind_chunk_q = np.empty(max(NCHI, 1), np.int64)
    self_chunk_batch = np.empty(max(N13, 1), np.int64)
    self_chunk_q = np.empty(max(N13, 1), np.int64)
    for idx, b in enumerate(batches):
        sl = slice(b["col"], b["col"] + b["qcount"])
        if b["kind"] == "ind":
            ind_chunk_batch[sl] = idx
            ind_chunk_q[sl] = np.arange(b["qcount"])
        else:
            self_chunk_batch[sl] = idx
            self_chunk_q[sl] = np.arange(b["qcount"])
    bat_hbase = np.array([b["hbase"] for b in batches], np.int64)
    bat_qc = np.array([b["qcount"] for b in batches], np.int64)

    # fold tiling: count-sorted dests; R_t = global max count per tile
    # (block 0 = self, read via plain DMA; fof covers r>=1 only)
    R_t = []
    for t in range(n_tiles):
        r = 1
        for c in range(n_cores):
            cs = core_counts_sorted[c]
            if t * 128 < len(cs):
                r = max(r, int(cs[t * 128]))
        R_t.append(r)
    colb = np.concatenate([[0], np.cumsum(np.maximum(np.array(R_t) - 1, 0))])
    NRF = int(colb[-1])      # fof columns (non-self)

    fold_ops = []
    t = 0
    while t < n_tiles:
        t0 = t
        ncols = 0
        while t < n_tiles and t - t0 < TMAX and ncols + R_t[t] <= QF:
            ncols += R_t[t]
            t += 1
        assert t > t0
        fold_ops.append(dict(t0=t0, ntiles=t - t0))

    meta = dict(N=N, S=S, NCHI=NCHI, N13=N13, batches=batches,
                chunk_meta=chunk_meta, n_H_rows=n_H_rows, ZH=ZH,
                R_t=R_t, NRF=NRF, n_tiles=n_tiles, S_pad=S_pad,
                fold_ops=fold_ops, n_segs=n_segs,
                self_chunk_batch=[int(x) for x in
                                  self_chunk_batch[:max(N13, 1)]],
                self_chunk_q=[int(x) for x in self_chunk_q[:max(N13, 1)]])

    # per-core data
    in_maps = []
    for c in range(n_cores):
        order = core_orders[c]
        rank = np.empty(S, np.int64)
        rank[order] = np.arange(S)

        # slot assignment per (seg, k) group; idx stream per chunk
        gidx_rows = np.full((NCHI, 128), -1, np.int64)  # abs table row
        all_rank = []
        all_hrow = []
        for k in ind_ks:
            dests, srcs = core_runs[c][k]
            if len(dests) == 0:
                continue
            seg_of = srcs // SEGR
            for seg in range(n_segs):
                m = seg_of == seg
                L = int(m.sum())
                if L == 0:
                    continue
                g0 = grp_start[(seg, k)]
                d_g = dests[m]
                s_g = srcs[m]
                j = np.arange(L)
                ci = g0 + j // 128
                p = j % 128
                gidx_rows[ci, p] = s_g
                all_rank.append(rank[d_g])
                bidx = ind_chunk_batch[ci]
                all_hrow.append(bat_hbase[bidx] + p * bat_qc[bidx]
                                + ind_chunk_q[ci])
        all_rank = np.concatenate(all_rank)
        all_hrow = np.concatenate(all_hrow)

        # pad slots -> any valid row of the chunk's segment (unreferenced)
        for ci in range(NCHI):
            seg = chunk_meta[ci][0]
            mpad = gidx_rows[ci] < 0
            gidx_rows[ci, mpad] = seg * SEGR

        # idx stream, int16 relative to segment, 16-wrapped + replicated:
        # op covers chunks [b.col+c0, +nch): idx j (chunk-local run) at
        # wrapped [16r + j%16, j//16]
        gidx = np.zeros((128, NCHI * 8), np.int16)
        for ci in range(NCHI):
            seg = chunk_meta[ci][0]
            rel = (gidx_rows[ci] - seg * SEGR).astype(np.int16)
            w = rel.reshape(8, 16).T          # [16, 8]
            gidx[:, ci * 8:(ci + 1) * 8] = np.tile(w, (8, 1))

        # fold offsets (non-self contributions, r>=1)
        o2 = np.argsort(all_rank, kind="stable")
        sr = all_rank[o2]
        sh = all_hrow[o2]
        grp_s = np.searchsorted(sr, np.arange(S))
        r_idx = np.arange(len(sr)) - grp_s[sr]
        t_of = sr // 128
        p_of = sr % 128
        col = colb[t_of] + r_idx
        assert (r_idx < np.array(R_t)[t_of] - 1).all(), "count excl self"
        fof = np.full((128, max(NRF, 1)), ZH, np.int32)
        fof[p_of, col] = sh

        # self image in dest-RANK order: chunk j slot p = dest order[j*128+p]
        d = np.arange(N13 * 128)
        src_local = np.where(d < S, order[np.minimum(d, S - 1)], 0)
        vals = table_bf[np.where(d < S, c * S + src_local, N)]  # [N13*128, C]
        self_img = np.ascontiguousarray(
            vals.reshape(N13, 128, C).transpose(1, 0, 2).reshape(128, N13 * C))

        in_maps.append({
            "table32": table32,
            "w2": w2,
            "gidx": gidx,
            "fof": fof,
            "self_img": self_img,
        })

    return in_maps, core_orders, meta


def build_program(n_cores, meta):
    NCHI, N13 = meta["NCHI"], meta["N13"]
    batches = meta["batches"]
    chunk_meta = meta["chunk_meta"]
    n_tiles = meta["n_tiles"]
    N = meta["N"]
    n_H_rows = meta["n_H_rows"]
    fold_ops = meta["fold_ops"]
    R_t = meta["R_t"]
    NRF = meta["NRF"]
    n_segs = meta["n_segs"]
    s_cb = meta["self_chunk_batch"]
    s_cq = meta["self_chunk_q"]
    colb = np.concatenate([[0], np.cumsum(np.maximum(np.array(R_t) - 1, 0))])

    nc = bacc.Bacc("TRN2", target_bir_lowering=False, debug=False,
                   num_devices=n_cores)

    table = nc.dram_tensor("table32", [N + 1, C], mybir.dt.float32,
                           kind="ExternalInput").ap()
    w_in = nc.dram_tensor("w2", [128, KVOL * 128], mybir.dt.bfloat16,
                          kind="ExternalInput").ap()
    gidx_in = nc.dram_tensor("gidx", [128, NCHI * 8], mybir.dt.int16,
                             kind="ExternalInput").ap()
    fof_in = nc.dram_tensor("fof", [128, max(NRF, 1)], mybir.dt.int32,
                            kind="ExternalInput").ap()
    self_in = nc.dram_tensor("self_img", [128, max(N13, 1) * C],
                             mybir.dt.bfloat16, kind="ExternalInput").ap()
    out = nc.dram_tensor("out_img", [128, n_tiles * C], mybir.dt.float32,
                         kind="ExternalOutput").ap()

    with tile.TileContext(nc) as tc, ExitStack() as ctx:
        dram = ctx.enter_context(tc.tile_pool(name="dram", bufs=1, space="DRAM"))
        h_dram = dram.tile([n_H_rows + 128, C], mybir.dt.bfloat16)

        wp = ctx.enter_context(tc.tile_pool(name="w", bufs=1))
        w_t = wp.tile([128, KVOL * 128], mybir.dt.bfloat16)
        nc.sync.dma_start(out=w_t[:], in_=w_in[:])
        ident = wp.tile([128, 128], mybir.dt.bfloat16)
        make_identity(nc, ident[:])
        identf = wp.tile([128, 128], mybir.dt.float32)
        nc.vector.tensor_copy(out=identf[:], in_=ident[:])
        gix = wp.tile([128, NCHI * 8], mybir.dt.int16)
        nc.sync.dma_start(out=gix[:], in_=gidx_in[:])
        fof = wp.tile([128, max(NRF, 1)], mybir.dt.int32)
        nc.sync.dma_start(out=fof[:], in_=fof_in[:])
        zt = wp.tile([128, C], mybir.dt.bfloat16)
        nc.vector.memset(zt[:], 0.0)
        nc.sync.dma_start(out=h_dram[n_H_rows:n_H_rows + 128, :], in_=zt[:])

        gp = ctx.enter_context(tc.tile_pool(name="G", bufs=4))
        xp = ctx.enter_context(tc.tile_pool(name="X", bufs=4))
        hp = ctx.enter_context(tc.tile_pool(name="H", bufs=3))
        psx = ctx.enter_context(tc.tile_pool(name="psx", bufs=3, space="PSUM"))
        psh = ctx.enter_context(tc.tile_pool(name="psh", bufs=3, space="PSUM"))

        def pair_pipeline(g_t, h_t, qc, ks, fdtype):
            npairs = qc // 2
            odd = qc % 2
            idf = identf if fdtype == mybir.dt.float32 else ident
            for g0 in range(0, npairs + odd, 4):
                gn = min(4, npairs + odd - g0)
                x_ps = psx.tile([128, 512], fdtype)
                if odd and g0 + gn == npairs + odd:
                    nc.vector.memset(x_ps[:], 0.0)
                for j in range(gn):
                    pr = g0 + j
                    if pr < npairs:
                        nc.tensor.transpose(
                            out=x_ps[:, j * 128:(j + 1) * 128],
                            in_=g_t[:, pr * 128:(pr + 1) * 128],
                            identity=idf[:])
                    else:
                        nc.tensor.transpose(
                            out=x_ps[0:64, j * 128:(j + 1) * 128],
                            in_=g_t[:, pr * 128:pr * 128 + 64],
                            identity=idf[:])
                x_t = xp.tile([128, 512], mybir.dt.bfloat16)
                nc.vector.tensor_copy(out=x_t[:, :gn * 128],
                                      in_=x_ps[:, :gn * 128])
                h_ps = psh.tile([128, 512], mybir.dt.float32)
                for j in range(gn):
                    pr = g0 + j
                    if pr < npairs:
                        k0, k1 = ks[2 * pr], ks[2 * pr + 1]
                        if k0 == k1:
                            nc.tensor.matmul(
                                out=h_ps[:, j * 128:(j + 1) * 128],
                                lhsT=x_t[:, j * 128:(j + 1) * 128],
                                rhs=w_t[:, k0 * 128:(k0 + 1) * 128],
                                start=True, stop=True)
                        else:
                            nc.tensor.matmul(
                                out=h_ps[:, j * 128:j * 128 + 64],
                                lhsT=x_t[0:64, j * 128:(j + 1) * 128],
                                rhs=w_t[0:64, k0 * 128:k0 * 128 + 64],
                                start=True, stop=True)
                            nc.tensor.matmul(
                                out=h_ps[:, j * 128 + 64:(j + 1) * 128],
                                lhsT=x_t[64:128, j * 128:(j + 1) * 128],
                                rhs=w_t[64:128, k1 * 128 + 64:(k1 + 1) * 128],
                                start=True, stop=True)
                    else:  # odd tail: single chunk in low half
                        k0 = ks[2 * pr]
                        nc.tensor.matmul(
                            out=h_ps[:, j * 128:j * 128 + 64],
                            lhsT=x_t[0:64, j * 128:(j + 1) * 128],
                            rhs=w_t[0:64, k0 * 128:k0 * 128 + 64],
                            start=True, stop=True)
                wcols = min(gn * 128, qc * 64 - g0 * 128)
                nc.scalar.activation(
                    h_t[:, g0 * 128:g0 * 128 + wcols],
                    h_ps[:, :wcols],
                    mybir.ActivationFunctionType.Copy)

        for b in batches:
            qc = b["qcount"]
            if b["kind"] == "ind":
                g_t = gp.tile([128, qc * C], mybir.dt.float32)
                for op in b["ops"]:
                    ni = op["nch"] * 128
                    seg = op["seg"]
                    seg_rows = min(SEGR, (N + 1) - seg * SEGR)
                    cbase = b["col"] + op["c0"]
                    nc.gpsimd.dma_gather(
                        out_ap=g_t[:, op["c0"] * C:(op["c0"] + op["nch"]) * C]
                            .rearrange("p (c e) -> p c e", c=op["nch"]),
                        in_ap=table[seg * SEGR:seg * SEGR + seg_rows, :],
                        idxs_ap=gix[:, cbase * 8:(cbase + op["nch"]) * 8],
                        num_idxs=ni, num_idxs_reg=ni, elem_size=C,
                        single_packet=ni <= 1024)
                ks = [chunk_meta[b["col"] + q][1] for q in range(qc)]
                fdtype = mybir.dt.float32
            else:
                g_t = gp.tile([128, qc * C], mybir.dt.bfloat16)
                nc.sync.dma_start(
                    out=g_t[:],
                    in_=self_in[:, b["col"] * C:(b["col"] + qc) * C])
                ks = [13] * qc
                fdtype = mybir.dt.bfloat16
            h_t = hp.tile([128, qc * C], mybir.dt.bfloat16)
            pair_pipeline(g_t, h_t, qc, ks, fdtype)
            nc.sync.dma_start(
                out=h_dram[b["hbase"]:b["hbase"] + 128 * qc, :].rearrange(
                    "(p q) c -> p (q c)", p=128),
                in_=h_t[:])

        fp = ctx.enter_context(tc.tile_pool(name="F", bufs=5))
        op_ = ctx.enter_context(tc.tile_pool(name="O", bufs=4))
        for fo in fold_ops:
            nt = fo["ntiles"]
            ncols = sum(R_t[fo["t0"] + i] for i in range(nt))
            f_t = fp.tile([128, ncols * C], mybir.dt.bfloat16)
            lc = 0
            for lt in range(nt):
                t = fo["t0"] + lt
                R = R_t[t]
                # block 0: self contribution, plain strided read
                bidx = s_cb[t]
                qcb = batches[bidx]["qcount"]
                hb = batches[bidx]["hbase"]
                qq = s_cq[t]
                nc.sync.dma_start(
                    out=f_t[:, lc * C:(lc + 1) * C],
                    in_=h_dram[hb:hb + 128 * qcb, :].rearrange(
                        "(p q) c -> p q c", p=128)[:, qq, :])
                # blocks 1..R-1: indirect per column
                for r in range(R - 1):
                    colx = int(colb[t]) + r
                    nc.gpsimd.indirect_dma_start(
                        out=f_t[:, (lc + 1 + r) * C:(lc + 2 + r) * C],
                        out_offset=None,
                        in_=h_dram[:],
                        in_offset=IndirectOffsetOnAxis(
                            ap=fof[:, colx:colx + 1], axis=0),
                    )
                lc += R
            ob = op_.tile([128, nt * C], mybir.dt.float32)
            lc = 0
            for lt in range(nt):
                R = R_t[fo["t0"] + lt]
                if R == 1:
                    nc.vector.tensor_copy(
                        out=ob[:, lt * C:(lt + 1) * C],
                        in_=f_t[:, lc * C:(lc + 1) * C])
                else:
                    nc.vector.tensor_reduce(
                        out=ob[:, lt * C:(lt + 1) * C],
                        in_=f_t[:, lc * C:(lc + R) * C].rearrange(
                            "p (r c) -> p c r", r=R),
                        axis=mybir.AxisListType.X,
                        op=mybir.AluOpType.add)
                lc += R
            nc.sync.dma_start(
                out=out[:, fo["t0"] * C:(fo["t0"] + nt) * C],
                in_=ob[:])

    nc.compile()
    return nc


def assemble_output(results, orders, meta, n_cores):
    S = meta["S"]
    N = meta["N"]
    n_tiles = meta["n_tiles"]
    out = np.empty((N, C), np.float32)
    for c in range(n_cores):
        img = results[c]["out_img"].reshape(128, n_tiles, C)
        rows = np.moveaxis(img, 0, 1).reshape(n_tiles * 128, C)
        out[c * S + orders[c]] = rows[:S]
    return out


LAST_EXEC_TIME_NS = None
_CACHE = {}


def kernel(feats, weight, kernel_map):
    """Full-input entry point: shard, run on 8 NeuronCores, unshard."""
    global LAST_EXEC_TIME_NS
    import os
    from concourse import bass_utils

    feats = np.asarray(feats)
    weight = np.asarray(weight)
    kernel_map = np.asarray(kernel_map)

    in_maps, orders, meta = host_prep(feats, weight, kernel_map, N_CORES)
    key = (meta["NCHI"], meta["N13"], meta["NRF"], tuple(meta["R_t"][:8]),
           len(meta["batches"]), len(meta["fold_ops"]))
    if key in _CACHE:
        nc = _CACHE[key]
    else:
        nc = build_program(N_CORES, meta)
        _CACHE[key] = nc

    trace = os.environ.get("BASS_KERNEL_TRACE", "0") == "1"
    res = bass_utils.run_bass_kernel_spmd(
        nc, in_maps, core_ids=list(range(N_CORES)), trace=trace)
    LAST_EXEC_TIME_NS = res.exec_time_ns
    return assemble_output(res.results, orders, meta, N_CORES)
